# revision 45
# baseline (speedup 1.0000x reference)
"""Position Attention Module (DANet) on 8 Trainium2 NeuronCores.

Reference computation (per batch b of 4):
  xf = x[b] : [C=512, N=4096]
  q = Wq@xf + bq : [64, N];  k = Wk@xf + bk : [64, N];  v = Wv@xf + bv : [512, N]
  scores[i,j] = q[:,i].k[:,j];  attn = softmax_j(scores)
  out[c,i] = alpha * sum_j v[c,j] attn[i,j]

Sharding: 2 cores per batch, each core owns half the query rows (i), full k/v.
Per-core x is pre-rolled on host so the owned i-half is always columns 0:2048.

Device design (v5 — late value projection, fused phases):
  out = alpha*(Wv xf + bv) attn^T = (alpha Wv)(xf attn^T) + alpha*bv
  (softmax rows sum to 1, so the bias contributes exactly alpha*bv[c]).
  The attention "values" are x itself (bf16, host-transposed); Wv applies
  AFTER attention on the [C, IH] result (64 matmuls/core vs the 128-matmul
  v-projection), leaving only the q/k projection in the x-streaming phase.
  - q/k projection f32r, psum drawn from the same pool as scores so
    attention i-tile 0 (j 0..15) interleaves with the x-paced kq chains.
  - scoresT [j,i] single K=64 f32r matmul; exp (no max-sub) -> bf16.
  - y[c',i] = sum_j xT[j,c'] e[j,i] accumulated fp32 in PSUM.
  - denominator matmul emitted BEFORE the last AV group so the DVE
    reciprocal overlaps it and psum banks recycle sooner.
  - y/den evicted bf16; out = WvT.T yb (+alpha*bv via ACT bias, c is the
    partition dim there).
  - pso bufs=4 (kq, scores, denom, Wv-stage) / pout bufs=4 (y accum);
    epool bufs=8 — measured optimum lookahead.
"""
import numpy as np
import ml_dtypes

BF = ml_dtypes.bfloat16

B, C, HW = 4, 512, 4096
CQ = 64
NCORES = 8
IH = HW // 2          # 2048 query rows per core
ITILE = 512           # i-tile (psum free dim)
NITILES = IH // ITILE # 4
JT = 128              # j-tile (contraction chunk for AV / scores lhsT cols)
NJT = HW // JT        # 32
JB = 512              # j-block for projections
XH = 2048             # x half width
NCC = C // 128        # 4 contraction chunks of 128 over C

_cache = {}


def _build():
    import concourse.bacc as bacc
    import concourse.tile as tile
    import concourse.mybir as mybir
    from concourse.bass_utils import run_bass_kernel_spmd

    f32 = mybir.dt.float32
    f32r = mybir.dt.float32r
    bf16 = mybir.dt.bfloat16
    AF = mybir.ActivationFunctionType

    nc = bacc.Bacc("TRN2", target_bir_lowering=False, debug=False)

    x_d = nc.dram_tensor("x", [C, HW], f32, kind="ExternalInput")
    xT_d = nc.dram_tensor("xT", [HW, C], bf16, kind="ExternalInput")
    wqk_d = nc.dram_tensor("wqk", [C, 128], f32, kind="ExternalInput")
    wvb_d = nc.dram_tensor("wvb", [C, C], bf16, kind="ExternalInput")
    bqk_d = nc.dram_tensor("bqk", [128, 1], f32, kind="ExternalInput")
    bvc_d = nc.dram_tensor("bvc", [128, NCC], f32, kind="ExternalInput")
    out_d = nc.dram_tensor("out", [C, IH], f32, kind="ExternalOutput")

    with tile.TileContext(nc) as tc:
        with (
            tc.tile_pool(name="const", bufs=1) as cpool,
            tc.tile_pool(name="kq", bufs=1) as kqpool,
            tc.tile_pool(name="vt", bufs=1) as vtpool,
            tc.tile_pool(name="xin", bufs=8) as xpool,
            tc.tile_pool(name="expp", bufs=8) as epool,
            tc.tile_pool(name="dnm", bufs=2) as dpool,
            tc.tile_pool(name="ost", bufs=8) as opool,
            tc.tile_pool(name="rows", bufs=2) as rpool,
            tc.tile_pool(name="pso", bufs=4, space="PSUM") as pso,
            tc.tile_pool(name="pout", bufs=4, space="PSUM") as pout,
        ):
            # --- DMA issue order (each issue slice costs ~630ns on Sync):
            # wqk -> x jb0 -> xT j0..7 -> wvb/biases -> x rest -> x half1
            # -> xT j8..31
            wqk = [cpool.tile([128, 128], f32r, tag=f"wqk{i}", name=f"wqk{i}") for i in range(NCC)]
            for cc in range(NCC):
                sl = slice(cc * 128, (cc + 1) * 128)
                nc.sync.dma_start(wqk[cc][:], wqk_d[sl, :].bitcast(f32r))
            xt = [[None] * NCC for _ in range(2)]
            for cc in range(NCC):
                t = xpool.tile([128, XH], f32r, tag="x", name=f"x0_{cc}")
                xt[0][cc] = t
            for cc in range(NCC):
                csl = slice(cc * 128, (cc + 1) * 128)
                nc.sync.dma_start(xt[0][cc][:, 0:JB], x_d[csl, 0:JB].bitcast(f32r))
            xts = [vtpool.tile([JT, C], bf16, tag=f"xt{j}", name=f"xt{j}") for j in range(NJT)]
            for j in range(2):
                nc.sync.dma_start(xts[j][:], xT_d[j * JT:(j + 1) * JT, :])
            bqk_c = cpool.tile([128, 1], f32, tag="bqkc")
            nc.sync.dma_start(bqk_c[:], bqk_d[:])
            for jb in range(1, XH // JB):
                jsl = slice(jb * JB, (jb + 1) * JB)
                for cc in range(NCC):
                    csl = slice(cc * 128, (cc + 1) * 128)
                    nc.sync.dma_start(xt[0][cc][:, jsl], x_d[csl, jsl].bitcast(f32r))
            for j in range(2, 8):
                nc.sync.dma_start(xts[j][:], xT_d[j * JT:(j + 1) * JT, :])
            for cc in range(NCC):
                csl = slice(cc * 128, (cc + 1) * 128)
                t = xpool.tile([128, XH], f32r, tag="x", name=f"x1_{cc}")
                nc.sync.dma_start(t[:], x_d[csl, XH:HW].bitcast(f32r))
                xt[1][cc] = t
            wvb = [cpool.tile([128, C], bf16, tag=f"wvb{i}", name=f"wvb{i}") for i in range(NCC)]
            for cc in range(NCC):
                sl = slice(cc * 128, (cc + 1) * 128)
                nc.sync.dma_start(wvb[cc][:], wvb_d[sl, :])
            bvc = cpool.tile([128, NCC], f32, tag="bvc")
            nc.sync.dma_start(bvc[:], bvc_d[:])
            for j in range(8, NJT):
                nc.sync.dma_start(xts[j][:], xT_d[j * JT:(j + 1) * JT, :])

            ones_sq = cpool.tile([128, 128], f32, tag="onessq")  # sum+bcast lhsT
            nc.vector.memset(ones_sq[:], 1.0)

            # k/q activations for scores, f32r single precision
            KHL = kqpool.tile([CQ, HW], f32r, tag="khl")
            QH = kqpool.tile([CQ, IH], f32r, tag="qh")

            def kq_chains(half):
                for jb in range(XH // JB):
                    lsl = slice(jb * JB, (jb + 1) * JB)       # within x tile
                    gof = half * XH + jb * JB                  # global j offset
                    gsl = slice(gof, gof + JB)
                    kqp = pso.tile([128, JB], f32, tag="sc")
                    if half == 0:
                        # packed q(rows 0:64) + k(rows 64:128) projection
                        for cc in range(NCC):
                            nc.tensor.matmul(kqp[:], wqk[cc][:], xt[half][cc][:, lsl],
                                             start=(cc == 0), stop=(cc == NCC - 1))
                        nc.scalar.activation(QH[:, gsl], kqp[0:CQ, :], AF.Identity,
                                             bias=bqk_c[0:CQ])
                        nc.scalar.activation(KHL[:, gsl], kqp[CQ:128, :], AF.Identity,
                                             bias=bqk_c[CQ:128])
                    else:
                        for cc in range(NCC):
                            nc.tensor.matmul(kqp[0:CQ, :], wqk[cc][:, CQ:128],
                                             xt[half][cc][:, lsl],
                                             start=(cc == 0), stop=(cc == NCC - 1))
                        nc.scalar.activation(KHL[:, gsl], kqp[0:CQ, :], AF.Identity,
                                             bias=bqk_c[CQ:128])

            def attn_js(it, ys, dnm, j0, j1):
                isl = slice(it * ITILE, (it + 1) * ITILE)
                dB = None
                for j in range(j0, j1):
                    jsl = slice(j * JT, (j + 1) * JT)
                    sp = pso.tile([JT, ITILE], f32, tag="sc")
                    nc.tensor.matmul(sp[:], KHL[:, jsl], QH[:, isl],
                                     start=True, stop=True)
                    et = epool.tile([JT, ITILE], bf16, tag="exp")
                    nc.scalar.activation(et[:], sp[:], AF.Exp)
                    if j == 0:
                        nc.vector.tensor_copy(dnm[:], et[:])
                    else:
                        nc.vector.tensor_add(dnm[:], dnm[:], et[:])
                    if j == NJT - 1:
                        # denominator broadcast emitted before the last AV
                        # group: the DVE reciprocal overlaps those matmuls
                        dB = pso.tile([128, ITILE], f32, tag="sc")
                        nc.tensor.matmul(dB[:], ones_sq[:].bitcast(f32r), dnm[:],
                                         start=True, stop=True)
                    for cc in range(NCC):
                        nc.tensor.matmul(
                            ys[cc][:], xts[j][:, cc * 128:(cc + 1) * 128], et[:],
                            start=(j == 0), stop=(j == NJT - 1))
                return dB

            def finish_it(it, ys, dB):
                isl = slice(it * ITILE, (it + 1) * ITILE)
                recipB = rpool.tile([128, ITILE], f32, tag="recipB")
                nc.vector.reciprocal_approx_fast(out=recipB[:], in_=dB[:])
                yb = []
                for cc in range(NCC):
                    t = opool.tile([128, ITILE], bf16, tag="yb")
                    nc.vector.tensor_mul(t[:], ys[cc][:], recipB[:])
                    yb.append(t)
                for co in range(NCC):
                    op2 = pso.tile([128, ITILE], f32, tag="sc")
                    for ci in range(NCC):
                        nc.tensor.matmul(op2[:], wvb[ci][:, co * 128:(co + 1) * 128],
                                         yb[ci][:], start=(ci == 0), stop=(ci == NCC - 1))
                    ot = opool.tile([128, ITILE], f32, tag="ot")
                    nc.scalar.activation(ot[:], op2[:], AF.Identity, bias=bvc[:, co:co + 1])
                    nc.sync.dma_start(out_d[co * 128:(co + 1) * 128, isl], ot[:])

            # it=0 (j 0..15) interleaves with the x-paced kq chains
            kq_chains(0)
            ys0 = [pout.tile([128, ITILE], f32, tag="op", name=f"ys0_{i}") for i in range(NCC)]
            dnm0 = dpool.tile([128, ITILE], f32r, tag="dn")
            attn_js(0, ys0, dnm0, 0, NJT // 2)
            kq_chains(1)
            dB0 = attn_js(0, ys0, dnm0, NJT // 2, NJT)
            # pipeline i-tile boundaries: the next tile's first scores are
            # emitted before the previous tile's Wv stage, filling the DVE
            # eviction window on the tensor queue
            pending = (0, ys0, dB0)
            for it in range(1, NITILES):
                ys = [pout.tile([128, ITILE], f32, tag="op", name=f"ys{it}_{i}") for i in range(NCC)]
                dnm = dpool.tile([128, ITILE], f32r, tag="dn")
                attn_js(it, ys, dnm, 0, 8)
                finish_it(*pending)
                dB = attn_js(it, ys, dnm, 8, NJT)
                pending = (it, ys, dB)
            finish_it(*pending)

    nc.compile()
    return nc, run_bass_kernel_spmd


def kernel(x, Wq, bq, Wk, bk, Wv, bv, alpha, trace=False, trace_kwargs=None):
    if "nc" not in _cache:
        _cache["nc"] = _build()
    nc, run_spmd = _cache["nc"]

    x = np.ascontiguousarray(np.asarray(x, dtype=np.float32)).reshape(B, C, HW)
    a = float(np.asarray(alpha, np.float32).reshape(-1)[0])
    wqk = np.ascontiguousarray(
        np.concatenate([np.asarray(Wq, np.float32).T, np.asarray(Wk, np.float32).T], axis=1))
    wvb = np.ascontiguousarray((np.asarray(Wv, np.float32).T * a).astype(BF))
    bqk = np.concatenate([np.asarray(bq, np.float32).reshape(CQ),
                          np.asarray(bk, np.float32).reshape(CQ)]).reshape(128, 1)
    bvc = np.ascontiguousarray(
        (np.asarray(bv, np.float32) * a).reshape(NCC, 128).T)

    in_maps = []
    for core in range(NCORES):
        b, ih = core // 2, core % 2
        xb = x[b]
        if ih:
            xb = np.ascontiguousarray(np.concatenate([xb[:, IH:], xb[:, :IH]], axis=1))
        xTb = np.ascontiguousarray(xb.T.astype(BF))
        in_maps.append({"x": xb, "xT": xTb, "wqk": wqk, "wvb": wvb,
                        "bqk": bqk, "bvc": bvc})

    kwargs = {}
    if trace:
        kwargs["trace"] = True
        kwargs.update(trace_kwargs or {})
    res = run_spmd(nc, in_maps, list(range(NCORES)), **kwargs)

    out = np.empty((B, C, HW), dtype=np.float32)
    for core in range(NCORES):
        b, ih = core // 2, core % 2
        out[b][:, ih * IH:(ih + 1) * IH] = res.results[core]["out"]
    if trace:
        return out.reshape(B, C, 64, 64), res
    return out.reshape(B, C, 64, 64)
